# revision 1
# baseline (speedup 1.0000x reference)
"""Causal self-attention (dense transformer block) on 8 Trainium2 NeuronCores.

Sharding: tensor-parallel over heads x data-parallel over batch.
  - 8 cores = 2 batch groups x 4 cores; each core owns 1 batch element and
    4 of the 16 heads (head_dim 64 -> 256 local channels).
  - Host pre-transposes x and the weight slices so the device never has to
    transpose activations (PE contracts along partitions).
  - Each core computes qkv projection for its heads, causal attention in
    "S^T" layout (scores[k, q], k on partitions), and its partial c_proj.
  - Host sums the 4 partials per batch and adds the bias terms.

Math notes:
  - k-bias and v-bias never enter the kernel: the k-bias contribution to the
    scores is constant along the softmax axis (cancels exactly), and the
    v-bias passes through softmax (rows sum to 1) and c_proj into a constant
    output offset w_proj @ b_v, added on host.
  - Softmax skips the max-subtraction pass: scores/8 have |.| <~ 3 for this
    distribution, exp cannot overflow, and the result is mathematically
    identical.
  - attV is computed with V augmented by a ones column, so the softmax
    denominators fall out of the same matmul (row 64 of the PSUM tile).
  - All matmuls run as float32r (fp32 truncated to ~fp22, full PE rate).

Scheduling notes:
  - Work is emitted per head-pair (V, Q^T/K^T, then attention) so the
    second pair's projection matmuls fill the PE gaps while the first
    pair's softmax runs on ScalarE (also keeps the PE HAM clock warm).
  - Softmax denominators are staged into a [16, 512] tile and inverted in
    ONE reciprocal op (DVE reciprocal costs ~6.3 cyc per free-dim element
    regardless of partition count; 16 separate [1,512] recips cost 53us).
  - attV PSUM tiles are released right after two cheap copies; the
    normalize-multiply happens later, in place, in SBUF.
"""

import numpy as np
from contextlib import ExitStack

import concourse.bass as bass
import concourse.tile as tile
from concourse import bacc, library_config, mybir
from concourse.bass_utils import run_bass_kernel_spmd

FP32 = mybir.dt.float32
FP32R = mybir.dt.float32r
AF = mybir.ActivationFunctionType

B, T_FULL, C = 2, 2048, 1024
H, D = 16, 64
NCORES = 8
CPG = 4          # cores per batch group
HPC = H // CPG   # heads per core = 4
HL = HPC * D     # local channels = 256
NQO = HL // 128  # head pairs per core = 2
CT = C // 128    # contraction tiles = 8


def _r(ap):
    return ap if ap.dtype == FP32R else ap.bitcast(FP32R)


def _nsplit(w):
    """Split width into matmul N-chunks at 512-aligned offsets (a matmul
    output may not cross a PSUM bank line)."""
    chunks = [512] * (w // 512)
    if w % 512:
        chunks.append(w % 512)
    return chunks


def build_bass(T=T_FULL):
    """Emit the SPMD Bass/Tile program for one core (same program, per-core
    data). T must be a multiple of 1024 (two halves per q-range, 512-chunks)."""
    assert T % 1024 == 0
    TT = T // 128          # t-tiles
    HALF = T // 2
    NCH = T // 512         # 512-chunks per head

    nc = bacc.Bacc("TRN2", target_bir_lowering=False, debug=False,
                   num_devices=NCORES)

    xT_d = nc.dram_tensor("xT", [C, T], FP32R, kind="ExternalInput")
    wqkvT_d = nc.dram_tensor("wqkvT", [C, 3 * HL], FP32R, kind="ExternalInput")
    bq_d = nc.dram_tensor("bq", [HL], FP32, kind="ExternalInput")
    wpT_d = nc.dram_tensor("wpT", [HL, C], FP32R, kind="ExternalInput")
    out_d = nc.dram_tensor("out", [T, C], FP32, kind="ExternalOutput")

    with tile.TileContext(nc) as tc, ExitStack() as ctx:
        xt = ctx.enter_context(tc.tile_pool(name="xt", bufs=CT))
        wq = ctx.enter_context(tc.tile_pool(name="wq", bufs=CT))
        qk = ctx.enter_context(tc.tile_pool(name="qk", bufs=2 * NQO))
        vv = ctx.enter_context(tc.tile_pool(name="vv", bufs=(TT + 3) // 4))
        es = ctx.enter_context(tc.tile_pool(name="es", bufs=3))
        yt = ctx.enter_context(tc.tile_pool(name="yt", bufs=NQO))
        ob = ctx.enter_context(tc.tile_pool(name="ob", bufs=3))
        bc = ctx.enter_context(tc.tile_pool(name="bc", bufs=2))
        sc = ctx.enter_context(tc.tile_pool(name="sc", bufs=1))
        # PSUM budget (8 banks): qkv/V 2x[128,512]=2, scores/proj 2x[128,1024]=4,
        # attV accumulators 2x[65,512]=2. Separate tags so the second pair's
        # qkv matmuls can fill PE gaps while attention waits on softmax.
        pq = ctx.enter_context(tc.tile_pool(name="pq", bufs=2, space="PSUM"))
        ss = ctx.enter_context(tc.tile_pool(name="ss", bufs=2, space="PSUM"))
        py = ctx.enter_context(tc.tile_pool(name="py", bufs=2, space="PSUM"))

        # ---- inputs -> SBUF (weights first: every qkv matmul needs them) ----
        wqs = []
        for c in range(CT):
            t_ = wq.tile([128, 3 * HL], FP32R, tag="wq", name="wtile")
            nc.gpsimd.dma_start(out=t_, in_=wqkvT_d[c * 128:(c + 1) * 128, :])
            wqs.append(t_)
        xts = []
        for c in range(CT):
            t_ = xt.tile([128, T], FP32R, tag="xt", name="xtile")
            # two queues, half-tile granularity: first matmuls start sooner
            nc.sync.dma_start(out=t_[:, 0:T // 2],
                              in_=xT_d[c * 128:(c + 1) * 128, 0:T // 2])
            nc.sync.dma_start(out=t_[:, T // 2:T],
                              in_=xT_d[c * 128:(c + 1) * 128, T // 2:T])
            xts.append(t_)
        bq_sb = sc.tile([128, NQO], FP32, tag="bq")
        nc.sync.dma_start(out=bq_sb, in_=bq_d.ap().rearrange("(j p) -> p j", p=128))

        # ones source for V's denominator column (ACT rounds fp32->fp32r)
        ones_sb = sc.tile([128, 4 * HPC], FP32, tag="ones")
        nc.gpsimd.memset(ones_sb, 1.0)
        vts = []
        for g in range((TT + 3) // 4):
            vt = vv.tile([128, 4, HPC, D + 1], FP32R, tag="vv", name="vtile")
            nc.scalar.copy(
                vt[:, :, :, D],
                ones_sb.rearrange("p (a b) -> p a b", a=4),
            )
            vts.append(vt)

        qk_tiles = [qk.tile([128, T], FP32R, tag="qk", name="qktile")
                    for _ in range(2 * NQO)]
        yts = [yt.tile([128, T], FP32R, tag="yt", name="ytile")
               for _ in range(NQO)]
        # softmax denominators: partition 32*cg, free column h*512.. ; unused
        # partitions memset so the whole-tile reciprocal is defined
        dstage = sc.tile([128, HPC * 512], FP32, tag="dstage")
        nc.gpsimd.memset(dstage, 1.0)

        # ---- V for all heads (N=256 keeps fp32r at full rate) ----
        for tt in range(TT):
            pv = pq.tile([128, 512], FP32, tag="pq", name="pv")
            for c in range(CT):
                nc.tensor.matmul(
                    pv[:, 0:HL],
                    _r(xts[c][:, tt * 128:(tt + 1) * 128]),
                    _r(wqs[c][:, 2 * HL:3 * HL]),
                    start=(c == 0), stop=(c == CT - 1),
                )
            nc.vector.tensor_copy(
                vts[tt // 4][:, tt % 4, :, 0:D],
                pv[:, 0:HL].rearrange("p (h d) -> p h d", h=HPC),
            )

        def emit_qk_pair(pair):
            for o in (pair, NQO + pair):
                col0 = o * 128 if o < NQO else HL + (o - NQO) * 128
                for tch in range(T // 512):
                    pt = pq.tile([128, 512], FP32, tag="pq", name="pqk")
                    for c in range(CT):
                        nc.tensor.matmul(
                            pt,
                            _r(wqs[c][:, col0:col0 + 128]),
                            _r(xts[c][:, tch * 512:(tch + 1) * 512]),
                            start=(c == 0), stop=(c == CT - 1),
                        )
                    dst = qk_tiles[o][:, tch * 512:(tch + 1) * 512]
                    if o < NQO:  # add q bias (per-partition)
                        nc.vector.tensor_scalar_add(dst, pt, bq_sb[:, o:o + 1])
                    else:
                        nc.vector.tensor_copy(dst, pt)

        def emit_attention_head(pair, h01):
            # the last head normalizes per chunk (costlier 1-lane recips, but
            # unblocks c_proj t-tiles as each 512-column chunk completes)
            last_head = (pair == NQO - 1 and h01 == 1)
            hb = 64 * h01
            h = 2 * pair + h01          # local head index 0..3
            qt = qk_tiles[pair]
            kt_tile = qk_tiles[NQO + pair]
            for half in range(2):
                q0, q1 = half * HALF, (half + 1) * HALF
                py_map = {}
                for kt in range(q1 // 128):
                    qa = max(kt * 128, q0)
                    w = q1 - qa
                    qa0 = (qa // 512) * 512
                    pt = ss.tile([128, 1024], FP32, tag="ss", name="pst")
                    off = 0
                    for cw in _nsplit(w):
                        nc.tensor.matmul(
                            pt[:, off:off + cw],
                            _r(kt_tile[hb:hb + 64, kt * 128:(kt + 1) * 128]),
                            _r(qt[hb:hb + 64, qa + off:qa + off + cw]),
                            start=True, stop=True,
                        )
                        off += cw
                    es_t = es.tile([128, 1024], FP32R, tag="es", name="estile")
                    nc.scalar.activation(
                        es_t[:, qa - qa0:qa - qa0 + w], pt[:, 0:w],
                        AF.Exp, scale=0.125,
                    )
                    if qa == kt * 128:
                        # causal mask: zero exp values where k > q in the
                        # diagonal block (gpsimd, SBUF, off the DVE/PSUM path)
                        nc.gpsimd.affine_select(
                            out=es_t[:, qa - qa0:qa - qa0 + 128],
                            in_=es_t[:, qa - qa0:qa - qa0 + 128],
                            compare_op=mybir.AluOpType.is_ge,
                            fill=0.0, base=0,
                            pattern=[[1, 128]], channel_multiplier=-1,
                        )
                    for cg in range(q0 // 512, q1 // 512):
                        if kt * 128 >= (cg + 1) * 512:
                            continue
                        if cg not in py_map:
                            py_map[cg] = py.tile([65, 512], FP32,
                                                 tag="py", name="pyt")
                        last_kt = min(q1 // 128, (cg + 1) * 4) - 1
                        # clip to causally-valid columns (q >= kt*128)
                        c0 = max(cg * 512, kt * 128)
                        nc.tensor.matmul(
                            py_map[cg][:, c0 - cg * 512:512],
                            _r(vts[kt // 4][:, kt % 4, h, :]),
                            _r(es_t[:, c0 - qa0:(cg + 1) * 512 - qa0]),
                            start=(kt == 0), stop=(kt == last_kt),
                        )
                        if kt == last_kt:
                            # stage unnormalized y + denominator row, then
                            # release the PSUM slot; normalize later in SBUF
                            py_t = py_map[cg]
                            nc.vector.tensor_copy(
                                yts[pair][hb:hb + 64,
                                          cg * 512:(cg + 1) * 512],
                                py_t[0:64, :],
                            )
                            nc.vector.tensor_copy(
                                dstage[32 * cg:32 * cg + 1,
                                       h * 512:(h + 1) * 512],
                                py_t[64:65, :])
                            if last_head:
                                dsl = dstage[32 * cg:32 * cg + 1,
                                             h * 512:(h + 1) * 512]
                                nc.vector.reciprocal(dsl, dsl)
                                rr = bc.tile([1, 512], FP32, tag="rr",
                                             name="rrow")
                                nc.sync.dma_start(out=rr, in_=dsl)
                                bc_t = bc.tile([128, 512], FP32, tag="bc",
                                               name="bct")
                                nc.gpsimd.partition_broadcast(bc_t, rr)
                                dst = yts[pair][hb:hb + 64,
                                                cg * 512:(cg + 1) * 512]
                                nc.vector.tensor_mul(dst, dst,
                                                     bc_t[hb:hb + 64, :])
            if last_head:
                return
            # head's denominators complete: one batched reciprocal (in
            # place), then broadcast + in-place scale — all of it overlaps
            # the next head's attention
            nc.vector.reciprocal(dstage[:, h * 512:(h + 1) * 512],
                                 dstage[:, h * 512:(h + 1) * 512])
            for cg in range(NCH):
                rr = bc.tile([1, 512], FP32, tag="rr", name="rrow")
                nc.sync.dma_start(
                    out=rr,
                    in_=dstage[32 * cg:32 * cg + 1, h * 512:(h + 1) * 512])
                bc_t = bc.tile([128, 512], FP32, tag="bc", name="bct")
                nc.gpsimd.partition_broadcast(bc_t, rr)
                dst = yts[pair][hb:hb + 64, cg * 512:(cg + 1) * 512]
                nc.vector.tensor_mul(dst, dst, bc_t[hb:hb + 64, :])

        for pair in range(NQO):
            emit_qk_pair(pair)
            for h01 in range(2):
                emit_attention_head(pair, h01)

        # ---- c_proj partial: out[t, co] = sum_ci y^T[ci, t] * wpT[ci, co] ----
        wps = []
        for i in range(NQO):
            t_ = xt.tile([128, C], FP32R, tag="xt", name="wptile")  # recycled
            nc.sync.dma_start(out=t_, in_=wpT_d[i * 128:(i + 1) * 128, :])
            wps.append(t_)
        for tt in range(TT):
            po = ss.tile([128, 1024], FP32, tag="ss", name="po")
            for s in range(2):
                for i in range(NQO):
                    nc.tensor.matmul(
                        po[:, s * 512:(s + 1) * 512],
                        _r(yts[i][:, tt * 128:(tt + 1) * 128]),
                        _r(wps[i][:, s * 512:(s + 1) * 512]),
                        start=(i == 0), stop=(i == NQO - 1),
                    )
            ot = ob.tile([128, C], FP32, tag="ob", name="otile")
            nc.scalar.copy(ot, po)  # ACT: DVE is the busier engine here
            nc.sync.dma_start(out=out_d[tt * 128:(tt + 1) * 128, :], in_=ot)

    nc.compile()  # bacc lowering: register allocation, library/ACT table loads
    return nc


_NC_CACHE = {}


def _get_nc(T=T_FULL):
    if T not in _NC_CACHE:
        _NC_CACHE[T] = build_bass(T)
    return _NC_CACHE[T]


def make_in_maps(x, w_attn, b_attn, w_proj, T=T_FULL):
    x = np.ascontiguousarray(np.asarray(x, np.float32))
    w_attn = np.asarray(w_attn, np.float32)
    b_attn = np.asarray(b_attn, np.float32)
    w_proj = np.asarray(w_proj, np.float32)
    xTs = [np.ascontiguousarray(x[b].T) for b in range(x.shape[0])]
    in_maps = []
    for core in range(NCORES):
        b, j = core // CPG, core % CPG
        r0 = j * HL
        wq_s = w_attn[r0:r0 + HL]
        wk_s = w_attn[C + r0:C + r0 + HL]
        wv_s = w_attn[2 * C + r0:2 * C + r0 + HL]
        in_maps.append({
            "xT": xTs[b],
            "wqkvT": np.ascontiguousarray(
                np.concatenate([wq_s, wk_s, wv_s], axis=0).T),
            "bq": np.ascontiguousarray(b_attn[r0:r0 + HL]),
            "wpT": np.ascontiguousarray(w_proj[:, r0:r0 + HL].T),
        })
    return in_maps


def run_device(x, w_attn, b_attn, w_proj, b_proj, T=T_FULL, **spmd_kwargs):
    nc = _get_nc(T)
    in_maps = make_in_maps(x, w_attn, b_attn, w_proj, T)
    res = run_bass_kernel_spmd(nc, in_maps, core_ids=list(range(NCORES)),
                               **spmd_kwargs)
    outs = [r["out"] for r in res.results]
    b_eff = (np.asarray(b_proj, np.float32)
             + np.asarray(w_proj, np.float32) @ np.asarray(b_attn, np.float32)[2 * C:])
    full = np.stack(
        [sum(outs[b * CPG:(b + 1) * CPG][1:], outs[b * CPG]) + b_eff
         for b in range(B)]
    ).astype(np.float32)
    return full, res


def kernel(x, w_attn, b_attn, w_proj, b_proj):
    out, _ = run_device(x, w_attn, b_attn, w_proj, b_proj)
    return out

